# revision 1
# baseline (speedup 1.0000x reference)
"""Trainium2 kernel for out[b] = P @ X[b] @ P.T  (basis-change of a density matrix).

P (7140, 1024) is a 0/1 matrix with exactly one 1 per column, so the op is a
pure scatter: out[b][rowmap[i], rowmap[j]] = X[b][i, j], zeros elsewhere.

Structure of the map (derived from P at runtime, asserted): input columns
group into 16 "lines" of 64; line l lands in the output as a 169-wide
cluster at base(l) (quadratic spacing), and within every cluster the 64
values sit at the same 15 runs of offsets. This lets the kernel:
  - keep a *packed* W tile [128 part, 2 row-groups, 16*169] in SBUF where the
    line dimension has constant stride 169,
  - perform the whole column scatter with ~15 wide DVE copies per line-half
    (access patterns may use different strides on input and output, so one op
    covers all lines x both row-groups x 4 channels),
  - let the 16 per-cluster store DMAs do the quadratic base(l) placement for
    free (DMA is bandwidth-bound, not op-bound).

Sharding: 8 cores, core c owns batch b=c//4 and input rows [256*q, 256*q+256)
(q=c%4), i.e. 256 mapped output rows plus a 1529-row chunk of the all-zero
output rows. Every core runs the identical program (SPMD) on its input slice.
Unwritten shard rows/columns are zero by the ExternalOutput contract
(run_bass_kernel_spmd pre-zeros / donates zero-initialized output buffers).
The host then unshards by placing each core's rows at their output positions.

Per-core program: GPSIMD memsets W (hidden under the X load), DVE does the
scatter copies (two line-halves so stores pipeline), SP issues loads/stores.
"""

import os
import numpy as np

_CORES = 8
_B = 2
_DIN = 1024
_DOUT = 7140
_RPC = _DIN // 4                          # 256 mapped rows per core
_ZPC = (_DOUT - _DIN) * _B // _CORES      # 1529 zero rows per core
_SHARD_ROWS = _RPC + _ZPC                 # 1785
_NL = 16                                  # lines
_LW = _DIN // _NL                         # 64 input cols per line


def _rowmap_from_P(P):
    """Output row index for each input row/col: the row of the single 1 in
    each column of P."""
    return np.argmax(np.asarray(P), axis=0).astype(np.int64)


def _cluster_structure(rowmap):
    """Split the map into 16 uniform line-clusters.

    Returns (bases, cw, runs) where bases[l] is the output column of cluster
    l, cw the common cluster width, and runs the list of
    (src_off, dst_off, length) copy runs shared by every cluster."""
    rm = rowmap.reshape(_NL, _LW)
    bases = rm[:, 0].copy()
    offs = rm - bases[:, None]
    if not (offs == offs[0]).all():
        raise ValueError("P does not have the expected uniform line structure")
    off0 = offs[0]
    if not ((np.diff(off0) >= 1).all() and off0[0] == 0):
        raise ValueError("cluster offsets not monotonic")
    cw = int(off0[-1]) + 1
    b = np.sort(bases)
    if (np.diff(b) < cw).any() or b[-1] + cw > _DOUT:
        raise ValueError("clusters overlap")
    runs = []
    s = 0
    for i in range(1, _LW + 1):
        if i == _LW or off0[i] != off0[i - 1] + 1:
            runs.append((s, int(off0[s]), i - s))
            s = i
    return [int(v) for v in bases], cw, runs


def _build_program(bases, cw, runs, iters=1):
    import concourse.bass as bass
    import concourse.mybir as mybir

    f32 = mybir.dt.float32
    nc = bass.Bass()
    x = nc.dram_tensor("x", [_RPC, _DIN], f32, kind="ExternalInput")
    out = nc.dram_tensor("out", [_SHARD_ROWS, _DOUT], f32, kind="ExternalOutput")

    # Pair layout: pair m = lines (2m, 2m+1), stored as ONE DMA spanning
    # [base(2m), base(2m+1)+cw) — the inter-cluster gap is materialized as
    # zeros in W (bigger descriptors beat fewer bytes on HW). Within W, pair
    # m is packed at woff[m]; line 2m at woff[m], line 2m+1 at woff[m]+s[m].
    npair = _NL // 2
    s = [bases[2 * m + 1] - bases[2 * m] for m in range(npair)]
    span = [s[m] + cw for m in range(npair)]
    woff = [0] * npair
    for m in range(1, npair):
        woff[m] = woff[m - 1] + span[m - 1]
    wtot = woff[-1] + span[-1]

    dpi = 16 * npair         # dma_sem incs per iter (8 pair stores)
    nb = min(2, iters)       # W/X buffers (ping-pong across iterations)

    # store ownership: SP pairs 0-3 (half0), ACT pairs 4-5, Pool pairs 6-7
    import os as _os
    _split = _os.environ.get("STORE_SPLIT", "4-4-0")
    if _split == "4-2-2":
        sp_pairs, act_pairs, pool_pairs = [0, 1, 2, 3], [4, 5], [6, 7]
    elif _split == "4-4-0":
        sp_pairs, act_pairs, pool_pairs = [0, 1, 2, 3], [4, 5, 6, 7], []
    elif _split == "8-0-0":
        sp_pairs, act_pairs, pool_pairs = list(range(8)), [], []
    elif _split == "3-3-2":
        sp_pairs, act_pairs, pool_pairs = [0, 1, 2], [3, 4, 5], [6, 7]
    elif _split == "alt":
        sp_pairs, act_pairs, pool_pairs = [0, 2, 4, 6], [1, 3, 5, 7], []
    else:
        raise ValueError(_split)

    from contextlib import ExitStack

    with ExitStack() as ctx:
        Ws = [
            ctx.enter_context(nc.sbuf_tensor(f"Wbuf{j}", [128, 2, wtot], f32))
            for j in range(nb)
        ]
        Xs = [
            ctx.enter_context(nc.sbuf_tensor(f"Xbuf{j}", [128, 2, _DIN], f32))
            for j in range(nb)
        ]
        dma_sem = ctx.enter_context(nc.semaphore("dma_sem"))
        dve_sem = ctx.enter_context(nc.semaphore("dve_sem"))
        ms_sem = ctx.enter_context(nc.semaphore("ms_sem"))
        l_sems = [
            [
                ctx.enter_context(nc.semaphore(f"l{j}_{h}_sem"))
                for h in range(2)
            ]
            for j in range(nb)
        ]
        block = ctx.enter_context(nc.Block())

        def store_pair(eng, j, m):
            return eng.dma_start(
                out=out[0:_RPC, bases[2 * m] : bases[2 * m] + span[m]].rearrange(
                    "(t p) c -> p t c", t=2
                ),
                in_=Ws[j][:, :, woff[m] : woff[m] + span[m]],
            ).then_inc(dma_sem, 16)

        def issue_load(eng, i):
            # two half-loads (lines 0-7 / 8-15) so the first pair copies can
            # start as soon as the first half arrives
            j = i % nb
            for h in range(2):
                eng.dma_start(
                    out=Xs[j][:, :, 512 * h : 512 * (h + 1)],
                    in_=x[:, 512 * h : 512 * (h + 1)].rearrange(
                        "(t p) c -> p t c", t=2
                    ),
                ).then_inc(l_sems[j][h], 16)

        # One-time zero fill (Pool memsets). Data runs are overwritten by
        # DVE every iteration; gap columns are never written again.
        # Quarter q of buffer j covers pairs (2q, 2q+1).
        ms_at = {}
        ms_order = []
        for half in range(2):
            for j in range(nb):
                for q in (2 * half, 2 * half + 1):
                    ms_order.append((j, q))
        for idx, (j, q) in enumerate(ms_order):
            ms_at[(j, q)] = idx + 1

        @block.gpsimd
        def _(gpsimd):
            for j, q in ms_order:
                lo_c = woff[2 * q]
                hi_c = woff[2 * q + 1] + span[2 * q + 1]
                gpsimd.memset(Ws[j][:, :, lo_c:hi_c], 0.0).then_inc(ms_sem, 1)
            for i in range(iters):
                j = i % nb
                for m in pool_pairs:
                    gpsimd.wait_ge(dve_sem, 8 * i + m + 1)
                    store_pair(gpsimd, j, m)

        @block.sync
        def _(sync):
            issue_load(sync, 0)
            if iters > 1:
                issue_load(sync, 1)
            for i in range(iters):
                j = i % nb
                for m in sp_pairs:
                    sync.wait_ge(dve_sem, 8 * i + m + 1)
                    store_pair(sync, j, m)
                if i + 2 < iters:
                    # prefetch iter i+2's X: its buffer's last reader is
                    # copies of iter i, all done once dve_sem >= 8(i+1)...
                    # pairs 0-3 done is NOT enough; guarded below by the
                    # scalar-issued guard being unnecessary: loads only
                    # overwrite X[i%nb], whose readers are iter i copies.
                    sync.wait_ge(dve_sem, 8 * (i + 1))
                    issue_load(sync, i + 2)
            sync.wait_ge(dma_sem, dpi * iters)

        @block.scalar
        def _(scalar):
            for i in range(iters):
                j = i % nb
                for m in act_pairs:
                    scalar.wait_ge(dve_sem, 8 * i + m + 1)
                    store_pair(scalar, j, m)

        def _mk_ap(T, offset, dims):
            ap = T[:].copy()
            ap.ap = mybir.VecI64Pair(dims)
            ap.offset = offset
            return ap

        @block.vector
        def _(vector):
            for i in range(iters):
                j = i % nb
                if i >= nb:
                    # W[j]'s data runs about to be overwritten: stores of
                    # iter i-nb must be done.
                    vector.wait_ge(dma_sem, dpi * (i - nb + 1))
                for m in range(npair):
                    if m % 4 == 0:
                        # pairs 0-3 read X lines 0-7 (load half 0); 4-7 half 1
                        vector.wait_ge(
                            l_sems[j][m // 4], 16 * (i // nb + 1)
                        )
                    if i < nb and m % 2 == 0:
                        # first use of this buffer: gap zeros must exist
                        vector.wait_ge(ms_sem, ms_at[(j, m // 2)])
                    for jj, (src, dst, ln) in enumerate(runs):
                        # one op covers both lines of the pair (stride s[m]
                        # on the W side, 64 on the X side), both row-groups
                        w_ap = _mk_ap(
                            Ws[j],
                            woff[m] + dst,
                            [[2 * wtot, 128], [wtot, 2], [s[m], 2], [1, ln]],
                        )
                        x_ap = _mk_ap(
                            Xs[j],
                            2 * m * _LW + src,
                            [[2 * _DIN, 128], [_DIN, 2], [_LW, 2], [1, ln]],
                        )
                        ins = vector.tensor_copy(w_ap, x_ap)
                        if jj == len(runs) - 1:
                            ins.then_inc(dve_sem, 1)

    return nc


def _shard_inputs(input_state):
    in_maps = []
    for c in range(_CORES):
        b, q = divmod(c, 4)
        sl = np.ascontiguousarray(
            input_state[b, _RPC * q : _RPC * (q + 1), :], dtype=np.float32
        )
        in_maps.append({"x": sl})
    return in_maps


def _unshard(results, rowmap):
    unmapped = np.setdiff1d(np.arange(_DOUT), rowmap)
    out = np.empty((_B, _DOUT, _DOUT), np.float32)
    for c in range(_CORES):
        b, q = divmod(c, 4)
        shard = results[c]["out"]
        out[b, rowmap[_RPC * q : _RPC * (q + 1)], :] = shard[:_RPC]
        out[b, unmapped[_ZPC * q : _ZPC * (q + 1)], :] = shard[_RPC:]
    return out


def kernel(input_state, P):
    from concourse.bass_utils import run_bass_kernel_spmd

    input_state = np.asarray(input_state)
    rowmap = _rowmap_from_P(P)
    bases, cw, runs = _cluster_structure(rowmap)
    nc = _build_program(bases, cw, runs)
    res = run_bass_kernel_spmd(
        nc, _shard_inputs(input_state), core_ids=list(range(_CORES)), trace=False
    )
    return _unshard(res.results, rowmap)

